# revision 12
# baseline (speedup 1.0000x reference)
"""Multi-head attention forward on 8 Trainium2 NeuronCores (Bass/Tile).

Problem: B=2, S=2048, HIDDEN=2048, HEADS=16, D_K=128, fp32 I/O,
mask all-ones (eval). torch-Linear convention: y = x @ W.T.

Sharding (batch x head-group): core c owns batch b=c//4 and heads
{4g..4g+3} where g=c%4 (concat dims [512g, 512g+512)).  Each core
reads ONLY its batch's q/k/v rows (12MB vs 48MB for pure
head-parallel) and its 512-dim weight slices, computes projections +
attention for its 4 heads, then ONE AllToAll per head among the 4
cores of its batch redistributes attention outputs token-major for a
sequence-sharded output projection (tokens [512g, 512g+512) of batch
b).  No all-reduce anywhere.

  - Phase A: project Q,K into the transposed [d, s] layout and V into
    the natural [s, d] layout, streaming HALF-rows ([128h, 1024t],
    half-major host layout so each read is one contiguous 256KB slab).
    Each projection = 2 half-passes of exactly 8 psum accumulators;
    every input byte is read once.  Weight chunks are interleaved
    into the row stream just-in-time (wq with Q-half0, wk with
    Q-half1, wv with K-half0, wo with V), all on the sync queue, so
    they never steal DMA-engine bandwidth from rows that gate the PE.
  - Phase B: flat software pipeline over (head l outer, q-block qb,
    kt-group g): scoresT tiles [k, q] via Kh-stationary matmuls into
    wide 3-bank PSUM regions (one wide EXP per region amortizes the
    ~352-cycle ACT fixed cost 3x), PV accumulation in the transposed
    layout, 1-group lookahead so the exp queue never drains.  Softmax
    denominator per (qb, l): running DVE adds of the wide exp tiles,
    3 folds on GPSIMD (SBUF bf16), then a PE ones-matmul partition
    reduce into the just-freed psB wide slot + DVE reciprocal —
    GPSIMD carries no partition reduces, so the per-head collectives
    can block its queue harmlessly.
  - After head l's last tail: AllGather #l (0.5MB ins, 4 ranks; a
    4-rank AllToAll is rejected by the mesh check) on the gpsimd
    queue, then 4 gathers of MY 512-token column slice of each
    member's block — the slice offset is pid%4-dependent, expressed
    as a partition_id-driven dynamic-offset DMA so the SPMD program
    stays identical across cores.  Heads 0-2's collectives hide
    completely under phase B compute.
  - Phase D: out = concat @ W_o.T for this core's 512 tokens, with
    the 16-chunk contraction ordered so the 12 chunks from heads 0-2
    run first — collective #3 hides under them.
Queue discipline: all input rows + weight chunks on sync (in
consumption order); scatters, collectives and gathers on gpsimd;
phase D stores split scalar/sync.  Phase A psum evacuation alternates
DVE/scalar so bank b is free ~2 casts after its stop-matmul.
Host side: pre-transpose/cast inputs to bf16 half-major, slice
weights per head-group, reassemble the per-core [512, 2048] fp32
token blocks into the full output.
"""

import math
from contextlib import ExitStack

import ml_dtypes
import numpy as np

import concourse.tile as tile
from concourse.ap import AP
from concourse import bacc, mybir
from concourse.bass_utils import run_bass_kernel_spmd

BF16 = mybir.dt.bfloat16
F32 = mybir.dt.float32
NPBF16 = ml_dtypes.bfloat16

HIDDEN = 2048
HEADS = 16
D_K = 128
B = 2
N_CORES = 8
N_GRP = 4                       # cores per batch (head groups)
HPC = HEADS // N_GRP            # heads per core (4)
DPC = HPC * D_K                 # concat cols per core (512)
NHT = HIDDEN // 128             # 16 hidden-dim 128-tiles
RG = [[0, 1, 2, 3], [4, 5, 6, 7]]


def _mha_kernel(ctx: ExitStack, tc: tile.TileContext, aps: dict, S: int):
    nc = tc.nc
    NKT = S // 128                   # seq 128-tiles (16)
    QBLK = 512
    NQB = S // QBLK                  # 4 q-blocks == token blocks == ranks
    SH = S // 2                      # half-row width (1024)
    scale = 1.0 / math.sqrt(D_K)
    # phase-B kt groups: (start_kt, count); alternating psum slots A/B
    GRP = [(0, 3), (3, 3), (6, 3), (9, 3), (12, 3), (15, 1)]
    NG = len(GRP)

    xT = {"q": aps["qT"], "k": aps["kT"], "v": aps["vT"]}  # [2*HIDDEN, SH]
    wT = {"q": aps["wqT"], "k": aps["wkT"], "v": aps["wvT"]}  # [128, NHT*DPC]
    woT = aps["woT"]                                # [128, NHT*HIDDEN]
    out = aps["out"]                                # [QBLK, HIDDEN] f32
    a2a_in = aps["a2a_in"]                          # per head [N_GRP*128, QBLK]
    a2a_out = aps["a2a_out"]                        # per head [N_GRP*128, QBLK]

    # ---- resident weights (pre-tiled on host) ----
    w_pool = ctx.enter_context(tc.tile_pool(name="wqkv", bufs=1))
    w_sb = {k: w_pool.tile([128, NHT * DPC], BF16, tag=f"w{k}", name=f"w{k}")
            for k in ("q", "k", "v")}
    wo_pool = ctx.enter_context(tc.tile_pool(name="wo", bufs=1))
    wo_sb = wo_pool.tile([128, NHT * HIDDEN], BF16, tag="wo")

    proj_pool = ctx.enter_context(tc.tile_pool(name="proj", bufs=1))
    qh_sb = proj_pool.tile([128, HPC * S], BF16, tag="qh")
    kh_sb = proj_pool.tile([128, HPC * S], BF16, tag="kh")
    vh_sb = proj_pool.tile([128, NKT * DPC], BF16, tag="vh")

    # ---- persistent SBUF pools ----
    xrow_pool = ctx.enter_context(tc.tile_pool(name="xrow", bufs=4))
    es_pool = ctx.enter_context(tc.tile_pool(name="es", bufs=6))
    acc_pool = ctx.enter_context(tc.tile_pool(name="acc", bufs=2))
    fld_pool = ctx.enter_context(tc.tile_pool(name="fld", bufs=1))
    rb_pool = ctx.enter_context(tc.tile_pool(name="rb", bufs=1))
    ao_pool = ctx.enter_context(tc.tile_pool(name="ao", bufs=2))
    osb_pool = ctx.enter_context(tc.tile_pool(name="osb", bufs=2))
    cc_sb = None  # allocated at phase B start, reusing wq's SBUF space

    # all-ones stationary tile for the PE partition-reduce of softmax
    # denominators
    ones_pool = ctx.enter_context(tc.tile_pool(name="ones", bufs=1))
    ones_sb = ones_pool.tile([128, 128], BF16, tag="ones")
    nc.vector.memset(ones_sb[:], 1.0)

    # weight chunk DMA emitters, interleaved into the row stream.
    # wq/wk/wv: 16 chunks of [128, DPC]; wo: 16 chunks of [128, HIDDEN].
    def w_chunk(k, ht):
        nc.sync.dma_start(out=w_sb[k][:, ht * DPC:(ht + 1) * DPC],
                          in_=wT[k][:, ht * DPC:(ht + 1) * DPC])

    def wo_chunk(ht):
        nc.sync.dma_start(out=wo_sb[:, ht * HIDDEN:(ht + 1) * HIDDEN],
                          in_=woT[:, ht * HIDDEN:(ht + 1) * HIDDEN])

    def phase_a():
        with tc.tile_pool(name="psA", bufs=8, space="PSUM") as psA:
            # Q / K: transposed [d, s] layout.  Per half: 8 accumulators
            # = 4 d-tiles x 2 s-blocks; every half-row read once.
            for proj, dst in (("q", qh_sb), ("k", kh_sb)):
                for hf in range(2):
                    ps = [psA.tile([128, QBLK], F32, tag="psA",
                                   name=f"ps{proj}{hf}_{i}") for i in range(8)]
                    for ht in range(NHT):
                        # JIT weight chunks one full pass ahead of use
                        if proj == "q" and hf == 0:
                            w_chunk("q", ht)
                        elif proj == "q" and hf == 1:
                            w_chunk("k", ht)
                        elif proj == "k" and hf == 0:
                            w_chunk("v", ht)
                        row = xrow_pool.tile([128, SH], BF16, tag="xrow")
                        nc.sync.dma_start(
                            out=row[:],
                            in_=xT[proj][hf * HIDDEN + ht * 128:
                                         hf * HIDDEN + (ht + 1) * 128, :])
                        for dt in range(HPC):
                            for sbi in range(2):
                                nc.tensor.matmul(
                                    ps[dt * 2 + sbi][:],
                                    lhsT=w_sb[proj][:, ht * DPC + dt * 128:
                                                    ht * DPC + (dt + 1) * 128],
                                    rhs=row[:, sbi * QBLK:(sbi + 1) * QBLK],
                                    start=(ht == 0), stop=(ht == NHT - 1))
                    for dt in range(HPC):
                        for sbi in range(2):
                            i = dt * 2 + sbi
                            sb = hf * 2 + sbi
                            dview = dst[:, dt * S + sb * QBLK:
                                        dt * S + (sb + 1) * QBLK]
                            # alternate DVE/scalar so banks free 2-wide
                            if i % 2 == 0:
                                nc.vector.tensor_copy(dview, ps[i][:])
                            else:
                                nc.scalar.copy(dview, ps[i][:])

            # V in natural [s, d] layout: stationary = vT seq-tile,
            # moving = W_v ht-block [128h, 512d] -> one bank per t-tile.
            for hf in range(2):
                psv = [psA.tile([128, DPC], F32, tag="psA",
                                name=f"psv{hf}_{i}") for i in range(8)]
                for ht in range(NHT):
                    vrow = xrow_pool.tile([128, SH], BF16, tag="xrow")
                    nc.sync.dma_start(
                        out=vrow[:],
                        in_=xT["v"][hf * HIDDEN + ht * 128:
                                    hf * HIDDEN + (ht + 1) * 128, :])
                    for sti in range(8):
                        nc.tensor.matmul(
                            psv[sti][:],
                            lhsT=vrow[:, sti * 128:(sti + 1) * 128],
                            rhs=w_sb["v"][:, ht * DPC:(ht + 1) * DPC],
                            start=(ht == 0), stop=(ht == NHT - 1))
                for sti in range(8):
                    st = hf * 8 + sti
                    dview = vh_sb[:, st * DPC:(st + 1) * DPC]
                    if sti % 2 == 0:
                        nc.vector.tensor_copy(dview, psv[sti][:])
                    else:
                        nc.scalar.copy(dview, psv[sti][:])

    coll_insts = []

    def phase_b():
        """Flat software pipeline over all (l, qb, g) groups: PE stream
        s(k), s(k+1), PV(k), s(k+2), PV(k+1), ... crosses iteration
        boundaries, so the scalar-engine EXP queue never drains and the
        PE never waits out a pipeline refill."""
        nonlocal cc_sb
        # cc reuses wq's SBUF space (same shape/dtype; wq is dead after
        # phase A - tile inserts the WAR dep on wq's readers)
        cc_sb = w_pool.tile([128, NHT * QBLK], BF16, tag="wq", name="cc")
        with tc.tile_pool(name="pssA", bufs=1, space="PSUM") as psA_pool, \
             tc.tile_pool(name="pssB", bufs=1, space="PSUM") as psB_pool, \
             tc.tile_pool(name="pspv", bufs=2, space="PSUM") as pv_pool:
            # l-outer: head l's AllToAll fires when its 4 q-blocks
            # finish - collectives 0..2 hide under phase B compute.
            groups = [(qb, l, g) for l in range(HPC) for qb in range(NQB)
                      for g in range(NG)]
            st = {}  # (qb, l) -> iteration state

            def state(qb, l):
                key = (qb, l)
                if key not in st:
                    st[key] = {
                        "pv": pv_pool.tile([128, QBLK], F32, tag="pv",
                                           name=f"pv{qb}_{l}"),
                        "ess": [None] * NG,
                        "acc": None,
                    }
                return st[key]

            def emit_scores(qb, l, g):
                it = state(qb, l)
                k0, n = GRP[g]
                pool = psA_pool if g % 2 == 0 else psB_pool
                w = pool.tile([128, 3 * QBLK], F32, tag="w",
                              name=f"wide{qb}_{l}_{g}")
                it[f"w{g}"] = w
                rhs_q = qh_sb[:, l * S + qb * QBLK: l * S + (qb + 1) * QBLK]
                for j in range(n):
                    kt = k0 + j
                    nc.tensor.matmul(
                        w[:, j * QBLK:(j + 1) * QBLK],
                        lhsT=kh_sb[:, l * S + kt * 128: l * S + (kt + 1) * 128],
                        rhs=rhs_q, start=True, stop=True)

            def emit_act(qb, l, g):
                it = state(qb, l)
                n = GRP[g][1]
                es = es_pool.tile([128, 3 * QBLK], BF16, tag="es",
                                  name=f"es{qb}_{l}_{g}")
                it["ess"][g] = es
                nc.scalar.activation(
                    es[:, :n * QBLK], it[f"w{g}"][:, :n * QBLK],
                    mybir.ActivationFunctionType.Exp, scale=scale)

            def emit_pv(qb, l, g):
                it = state(qb, l)
                k0, n = GRP[g]
                for j in range(n):
                    kt = k0 + j
                    nc.tensor.matmul(
                        it["pv"][:],
                        lhsT=vh_sb[:, kt * DPC + l * 128:
                                   kt * DPC + (l + 1) * 128],
                        rhs=it["ess"][g][:, j * QBLK:(j + 1) * QBLK],
                        start=(kt == 0), stop=(kt == NKT - 1))

            def emit_add(qb, l, g):
                it = state(qb, l)
                if g == 0 or g == 5:
                    return
                a = acc_pool.tile([128, 3 * QBLK], BF16, tag="acc",
                                  name=f"acc{qb}_{l}_{g}")
                if g == 1:
                    nc.vector.tensor_add(a[:], it["ess"][0][:],
                                         it["ess"][1][:])
                else:
                    nc.vector.tensor_add(a[:], it["acc"][:], it["ess"][g][:])
                it["acc"] = a

            def emit_tail(qb, l):
                # fold 1536 -> 512 (+ last group) on GPSIMD, PE
                # ones-matmul partition reduce into the freed psB wide
                # slot, DVE reciprocal, normalize, scatter to a2a_in.
                it = state(qb, l)
                a4 = it["acc"]
                f1 = fld_pool.tile([128, QBLK], BF16, tag="f1")
                nc.gpsimd.tensor_add(f1[:], a4[:, 0:QBLK],
                                     a4[:, QBLK:2 * QBLK])
                f2 = fld_pool.tile([128, QBLK], BF16, tag="f2")
                nc.gpsimd.tensor_add(f2[:], f1[:], a4[:, 2 * QBLK:3 * QBLK])
                f3 = fld_pool.tile([128, QBLK], BF16, tag="f3")
                nc.gpsimd.tensor_add(f3[:], f2[:], it["ess"][5][:, 0:QBLK])
                rbp = psB_pool.tile([128, 3 * QBLK], F32, tag="w",
                                    name=f"rbp{qb}_{l}")
                nc.tensor.matmul(rbp[:, 0:QBLK], lhsT=ones_sb[:],
                                 rhs=f3[:], start=True, stop=True)
                rb = rb_pool.tile([128, QBLK], F32, tag="rb")
                nc.vector.reciprocal_approx_fast(rb[:], rbp[:, 0:QBLK])
                ao = ao_pool.tile([128, QBLK], BF16, tag="ao")
                nc.vector.tensor_mul(ao[:], it["pv"][:], rb[:])
                nc.gpsimd.dma_start(
                    out=a2a_in[l][:, qb * QBLK:(qb + 1) * QBLK], in_=ao[:])

            def emit_coll(l, tok_off):
                # end-of-head AllGather among this batch's 4 cores; the
                # trigger blocks the gpsimd queue until completion, which
                # only delays head l+1's (non-critical) fold work.
                coll = nc.gpsimd.collective_compute(
                    "AllGather", mybir.AluOpType.bypass,
                    replica_groups=RG,
                    ins=[a2a_in[l][:, :]], outs=[a2a_out[l][:, :]])
                coll_insts.append(coll)
                for gi in range(N_GRP):
                    dt = gi * HPC + l
                    src = AP(a2a_out[l].tensor,
                             tok_off + gi * 128 * S, [[S, 128], [1, QBLK]])
                    dma = nc.gpsimd.dma_start(
                        out=cc_sb[:, dt * QBLK:(dt + 1) * QBLK], in_=src)
                    tile.add_dep_helper(dma.ins, coll.ins,
                                        reason="a2a_out after collective")

            # my token-block column offset inside each AllGather block:
            # (pid % N_GRP) * QBLK, loaded once on the gpsimd engine
            tok_off = (nc.gpsimd.partition_id() % N_GRP) * QBLK

            emit_scores(*groups[0])
            for k, grp in enumerate(groups):
                if k + 1 < len(groups):
                    emit_scores(*groups[k + 1])
                emit_act(*grp)
                emit_pv(*grp)
                emit_add(*grp)
                if grp[2] == NG - 1:
                    emit_tail(grp[0], grp[1])
                    if grp[0] == NQB - 1:
                        emit_coll(grp[1], tok_off)

    # ================= Phase D: output projection ======================
    def phase_d():
        # contraction chunks from heads 0..2 first: collective #3 hides
        # under their 12 accumulation steps (psum accumulate commutes).
        DT_ORDER = [dt for l in range(HPC) for dt in (l, 4 + l, 8 + l, 12 + l)]
        DT_ORDER = ([dt for dt in DT_ORDER if dt % HPC != HPC - 1]
                    + [dt for dt in DT_ORDER if dt % HPC == HPC - 1])
        OBLK = 512
        NOB = HIDDEN // OBLK
        with tc.tile_pool(name="pso", bufs=8, space="PSUM") as pso_pool:
            for p in range(2):
                pso = [pso_pool.tile([128, OBLK], F32, tag="pso",
                                     name=f"pso{p}_{i}") for i in range(8)]
                for di, dt in enumerate(DT_ORDER):
                    for ti in range(2):
                        tt = p * 2 + ti
                        lhs = cc_sb[:, dt * QBLK + tt * 128:
                                    dt * QBLK + (tt + 1) * 128]
                        for ot in range(NOB):
                            nc.tensor.matmul(
                                pso[ti * NOB + ot][:], lhsT=lhs,
                                rhs=wo_sb[:, dt * HIDDEN + ot * OBLK:
                                          dt * HIDDEN + (ot + 1) * OBLK],
                                start=(di == 0), stop=(di == NHT - 1))
                for ti in range(2):
                    tt = p * 2 + ti
                    for ot in range(NOB):
                        osb = osb_pool.tile([128, OBLK], F32, tag="osb")
                        # alternate engines so the end-of-kernel drain
                        # runs two-wide.
                        if ot % 2 == 0:
                            nc.vector.tensor_copy(osb[:], pso[ti * NOB + ot][:])
                            eng = nc.scalar
                        else:
                            nc.scalar.copy(osb[:], pso[ti * NOB + ot][:])
                            eng = nc.sync
                        eng.dma_start(
                            out=out[tt * 128:(tt + 1) * 128,
                                    ot * OBLK:(ot + 1) * OBLK],
                            in_=osb[:])

    phase_a()
    phase_b()
    phase_d()


def build_nc(S: int):
    nc = bacc.Bacc("TRN2", target_bir_lowering=False, debug=False,
                   enable_asserts=False, num_devices=N_CORES)
    SH = S // 2
    QBLK = 512
    aps = {
        # half-major: [2 halves, HIDDEN, S/2] flattened - contiguous
        # 256KB half-row slabs for streaming reads
        "qT": nc.dram_tensor("qT", [2 * HIDDEN, SH], BF16,
                             kind="ExternalInput").ap(),
        "kT": nc.dram_tensor("kT", [2 * HIDDEN, SH], BF16,
                             kind="ExternalInput").ap(),
        "vT": nc.dram_tensor("vT", [2 * HIDDEN, SH], BF16,
                             kind="ExternalInput").ap(),
        "wqT": nc.dram_tensor("wqT", [128, NHT * DPC], BF16,
                              kind="ExternalInput").ap(),
        "wkT": nc.dram_tensor("wkT", [128, NHT * DPC], BF16,
                              kind="ExternalInput").ap(),
        "wvT": nc.dram_tensor("wvT", [128, NHT * DPC], BF16,
                              kind="ExternalInput").ap(),
        "woT": nc.dram_tensor("woT", [128, NHT * HIDDEN], BF16,
                              kind="ExternalInput").ap(),
        "out": nc.dram_tensor("out", [QBLK, HIDDEN], F32,
                              kind="ExternalOutput").ap(),
        "a2a_in": [nc.dram_tensor(f"a2a_in{l}", [128, S],
                                  BF16).ap() for l in range(HPC)],
        "a2a_out": [nc.dram_tensor(f"a2a_out{l}", [N_GRP * 128, S],
                                   BF16).ap() for l in range(HPC)],
    }
    with tile.TileContext(nc) as tc:
        with ExitStack() as ctx:
            _mha_kernel(ctx, tc, aps, S)
    nc.compile()
    return nc


_NC_CACHE: dict = {}


def _tile_weight(w_slice_T):
    """[H, D] -> [128, (H//128)*D] with 128-row tiles laid out consecutively."""
    H, D = w_slice_T.shape
    return np.ascontiguousarray(
        w_slice_T.reshape(H // 128, 128, D).transpose(1, 0, 2).reshape(
            128, (H // 128) * D))


def _half_major(x):
    """[S, H] -> [2*H, S/2]: transpose then split S into 2 contiguous halves."""
    S, H = x.shape
    return np.ascontiguousarray(
        x.T.reshape(H, 2, S // 2).transpose(1, 0, 2).reshape(
            2 * H, S // 2)).astype(NPBF16)


def make_in_maps(q, k, v, w_q, w_k, w_v, w_o):
    """Host-side shard/cast. Returns per-core input dicts."""
    qT = [_half_major(q[b]) for b in range(B)]
    kT = [_half_major(k[b]) for b in range(B)]
    vT = [_half_major(v[b]) for b in range(B)]
    woT = _tile_weight(np.ascontiguousarray(w_o.T).astype(NPBF16))
    wslices = []
    for g in range(N_GRP):
        d0 = g * DPC
        wslices.append({
            "wqT": _tile_weight(
                np.ascontiguousarray(w_q[d0:d0 + DPC, :].T).astype(NPBF16)),
            "wkT": _tile_weight(
                np.ascontiguousarray(w_k[d0:d0 + DPC, :].T).astype(NPBF16)),
            "wvT": _tile_weight(
                np.ascontiguousarray(w_v[d0:d0 + DPC, :].T).astype(NPBF16)),
        })
    in_maps = []
    for c in range(N_CORES):
        b, g = divmod(c, N_GRP)
        m = {"qT": qT[b], "kT": kT[b], "vT": vT[b], "woT": woT}
        m.update(wslices[g])
        in_maps.append(m)
    return in_maps


def kernel(q, k, v, mask, w_q, w_k, w_v, w_o, _trace=False):
    q = np.asarray(q, np.float32)
    k = np.asarray(k, np.float32)
    v = np.asarray(v, np.float32)
    mask = np.asarray(mask)
    w_q = np.asarray(w_q, np.float32)
    w_k = np.asarray(w_k, np.float32)
    w_v = np.asarray(w_v, np.float32)
    w_o = np.asarray(w_o, np.float32)
    S = q.shape[1]

    if not np.all(mask != 0):
        # General-mask fallback (never hit for the eval problem: mask is
        # all ones).  Computed on host for correctness.
        return _numpy_reference(q, k, v, mask, w_q, w_k, w_v, w_o)

    if S not in _NC_CACHE:
        _NC_CACHE[S] = build_nc(S)
    nc = _NC_CACHE[S]

    in_maps = make_in_maps(q, k, v, w_q, w_k, w_v, w_o)
    res = run_bass_kernel_spmd(nc, in_maps, core_ids=list(range(N_CORES)),
                               trace=_trace)

    QBLK = 512
    out = np.empty((B, S, HIDDEN), np.float32)
    for c in range(N_CORES):
        b, g = divmod(c, N_GRP)
        out[b, g * QBLK:(g + 1) * QBLK, :] = res.results[c]["out"]
    if _trace:
        return out, res
    return out


def _numpy_reference(q, k, v, mask, w_q, w_k, w_v, w_o):
    Bn, S, H = q.shape
    dk = H // HEADS

    def split_heads(x, w):
        y = x @ w.T
        return y.reshape(Bn, S, HEADS, dk).transpose(0, 2, 1, 3)

    qh = split_heads(q, w_q)
    kh = split_heads(k, w_k)
    vh = split_heads(v, w_v)
    s = np.einsum("bhqd,bhkd->bhqk", qh, kh) / np.sqrt(np.float32(dk))
    s = np.where(mask[:, None, :, :] == 0, np.float32(-1e9), s)
    s = s - s.max(-1, keepdims=True)
    e = np.exp(s)
    a = e / e.sum(-1, keepdims=True)
    o = np.einsum("bhqk,bhkd->bhqd", a, vh)
    o = o.transpose(0, 2, 1, 3).reshape(Bn, S, H)
    return (o @ w_o.T).astype(np.float32)


# revision 14
# speedup vs baseline: 1.2699x; 1.2699x over previous
"""Multi-head attention forward on 8 Trainium2 NeuronCores (Bass/Tile).

Problem: B=2, S=2048, HIDDEN=2048, HEADS=16, D_K=128, fp32 I/O,
mask all-ones (eval). torch-Linear convention: y = x @ W.T.

Sharding (batch x head-group): core c owns batch b=c//4 and heads
{4g..4g+3} where g=c%4 (concat dims [512g, 512g+512)).  Each core
reads ONLY its batch's q/k/v rows (12MB vs 48MB for pure
head-parallel) and its 512-dim weight slices, computes projections +
attention for its 4 heads, then ONE AllToAll per head among the 4
cores of its batch redistributes attention outputs token-major for a
sequence-sharded output projection (tokens [512g, 512g+512) of batch
b).  No all-reduce anywhere.

  - Phase A: project Q,K into the transposed [d, s] layout and V into
    the natural [s, d] layout, streaming HALF-rows ([128h, 1024t],
    half-major host layout so each read is one contiguous 256KB slab).
    Each projection = 2 half-passes of exactly 8 psum accumulators;
    every input byte is read once.  Weight chunks are interleaved
    into the row stream just-in-time (wq with Q-half0, wk with
    Q-half1, wv with K-half0, wo with V), all on the sync queue, so
    they never steal DMA-engine bandwidth from rows that gate the PE.
  - Phase B: flat software pipeline over (head l outer, q-block qb,
    kt-group g): scoresT tiles [k, q] via Kh-stationary matmuls into
    wide 3-bank PSUM regions (one wide EXP per region amortizes the
    ~352-cycle ACT fixed cost 3x), PV accumulation in the transposed
    layout, 1-group lookahead so the exp queue never drains.  Softmax
    denominator per (qb, l): running DVE adds of the wide exp tiles,
    3 folds on GPSIMD (SBUF bf16), then a PE ones-matmul partition
    reduce into the just-freed psB wide slot + DVE reciprocal —
    GPSIMD carries no partition reduces, so the per-head collectives
    can block its queue harmlessly.
  - After head l's last tail: AllGather #l (0.5MB ins, 4 ranks; a
    4-rank AllToAll is rejected by the mesh check) on the gpsimd
    queue, then 4 gathers of MY 512-token column slice of each
    member's block — the slice offset is pid%4-dependent, expressed
    as a partition_id-driven dynamic-offset DMA so the SPMD program
    stays identical across cores.  Heads 0-2's collectives hide
    completely under phase B compute.
  - Phase D: out = concat @ W_o.T for this core's 512 tokens, with
    the 16-chunk contraction ordered so the 12 chunks from heads 0-2
    run first — collective #3 hides under them.
Queue discipline: all input rows + weight chunks on sync (in
consumption order); scatters, collectives and gathers on gpsimd;
phase D stores split scalar/sync.  Phase A psum evacuation alternates
DVE/scalar so bank b is free ~2 casts after its stop-matmul.
Host side: pre-transpose/cast inputs to bf16 half-major, slice
weights per head-group, reassemble the per-core [512, 2048] fp32
token blocks into the full output.
"""

import math
from contextlib import ExitStack

import ml_dtypes
import numpy as np

import concourse.tile as tile
from concourse.ap import AP
from concourse import bacc, mybir
from concourse.bass_utils import run_bass_kernel_spmd

BF16 = mybir.dt.bfloat16
F32 = mybir.dt.float32
NPBF16 = ml_dtypes.bfloat16

HIDDEN = 2048
HEADS = 16
D_K = 128
B = 2
N_CORES = 8
N_GRP = 4                       # cores per batch (head groups)
HPC = HEADS // N_GRP            # heads per core (4)
DPC = HPC * D_K                 # concat cols per core (512)
NHT = HIDDEN // 128             # 16 hidden-dim 128-tiles
RG = [[0, 1, 2, 3], [4, 5, 6, 7]]


def _mha_kernel(ctx: ExitStack, tc: tile.TileContext, aps: dict, S: int):
    nc = tc.nc
    NKT = S // 128                   # seq 128-tiles (16)
    QBLK = 512
    NQB = S // QBLK                  # 4 q-blocks == token blocks == ranks
    SH = S // 2                      # half-row width (1024)
    scale = 1.0 / math.sqrt(D_K)
    # phase-B kt groups: (start_kt, count); alternating psum slots A/B
    GRP = [(0, 3), (3, 3), (6, 3), (9, 3), (12, 3), (15, 1)]
    NG = len(GRP)

    xT = {"q": aps["qT"], "k": aps["kT"], "v": aps["vT"]}  # [2*HIDDEN, SH]
    wT = {"q": aps["wqT"], "k": aps["wkT"], "v": aps["wvT"]}  # [128, NHT*DPC]
    woT = aps["woT"]                                # [128, NHT*HIDDEN]
    out = aps["out"]                                # [QBLK, HIDDEN] f32
    a2a_in = aps["a2a_in"]                          # per head [N_GRP*128, QBLK]
    a2a_out = aps["a2a_out"]                        # per head [N_GRP*128, QBLK]

    # ---- resident weights (pre-tiled on host) ----
    w_pool = ctx.enter_context(tc.tile_pool(name="wqkv", bufs=1))
    w_sb = {k: w_pool.tile([128, NHT * DPC], BF16, tag=f"w{k}", name=f"w{k}")
            for k in ("q", "k", "v")}
    wo_pool = ctx.enter_context(tc.tile_pool(name="wo", bufs=1))
    wo_sb = wo_pool.tile([128, NHT * HIDDEN], BF16, tag="wo")

    proj_pool = ctx.enter_context(tc.tile_pool(name="proj", bufs=1))
    qh_sb = proj_pool.tile([128, HPC * S], BF16, tag="qh")
    kh_sb = proj_pool.tile([128, HPC * S], BF16, tag="kh")
    vh_sb = proj_pool.tile([128, NKT * DPC], BF16, tag="vh")

    # ---- persistent SBUF pools ----
    xrow_pool = ctx.enter_context(tc.tile_pool(name="xrow", bufs=4))
    es_pool = ctx.enter_context(tc.tile_pool(name="es", bufs=6))
    acc_pool = ctx.enter_context(tc.tile_pool(name="acc", bufs=2))
    fld_pool = ctx.enter_context(tc.tile_pool(name="fld", bufs=1))
    rb_pool = ctx.enter_context(tc.tile_pool(name="rb", bufs=1))
    ao_pool = ctx.enter_context(tc.tile_pool(name="ao", bufs=2))
    osb_pool = ctx.enter_context(tc.tile_pool(name="osb", bufs=2))
    cc_sb = None  # allocated at phase B start, reusing wq's SBUF space

    # all-ones stationary tile for the PE partition-reduce of softmax
    # denominators
    ones_pool = ctx.enter_context(tc.tile_pool(name="ones", bufs=1))
    ones_sb = ones_pool.tile([128, 128], BF16, tag="ones")
    nc.vector.memset(ones_sb[:], 1.0)

    # weight chunk DMA emitters, interleaved into the row stream.
    # wq/wk/wv: 16 chunks of [128, DPC]; wo: 16 chunks of [128, HIDDEN].
    def w_chunk(k, ht):
        nc.sync.dma_start(out=w_sb[k][:, ht * DPC:(ht + 1) * DPC],
                          in_=wT[k][:, ht * DPC:(ht + 1) * DPC])

    def wo_chunk(ht):
        nc.sync.dma_start(out=wo_sb[:, ht * HIDDEN:(ht + 1) * HIDDEN],
                          in_=woT[:, ht * HIDDEN:(ht + 1) * HIDDEN])

    def phase_a():
        with tc.tile_pool(name="psA", bufs=8, space="PSUM") as psA:
            # Q / K: transposed [d, s] layout.  Per half: 8 accumulators
            # = 4 d-tiles x 2 s-blocks; every half-row read once.
            for proj, dst in (("q", qh_sb), ("k", kh_sb)):
                for hf in range(2):
                    ps = [psA.tile([128, QBLK], F32, tag="psA",
                                   name=f"ps{proj}{hf}_{i}") for i in range(8)]
                    for ht in range(NHT):
                        # JIT weight chunks one full pass ahead of use
                        if proj == "q" and hf == 0:
                            w_chunk("q", ht)
                        elif proj == "q" and hf == 1:
                            w_chunk("k", ht)
                        elif proj == "k" and hf == 0:
                            w_chunk("v", ht)
                        row = xrow_pool.tile([128, SH], BF16, tag="xrow")
                        nc.sync.dma_start(
                            out=row[:],
                            in_=xT[proj][hf * HIDDEN + ht * 128:
                                         hf * HIDDEN + (ht + 1) * 128, :])
                        for dt in range(HPC):
                            for sbi in range(2):
                                nc.tensor.matmul(
                                    ps[dt * 2 + sbi][:],
                                    lhsT=w_sb[proj][:, ht * DPC + dt * 128:
                                                    ht * DPC + (dt + 1) * 128],
                                    rhs=row[:, sbi * QBLK:(sbi + 1) * QBLK],
                                    start=(ht == 0), stop=(ht == NHT - 1))
                    for dt in range(HPC):
                        for sbi in range(2):
                            i = dt * 2 + sbi
                            sb = hf * 2 + sbi
                            dview = dst[:, dt * S + sb * QBLK:
                                        dt * S + (sb + 1) * QBLK]
                            # alternate DVE/scalar so banks free 2-wide
                            if i % 2 == 0:
                                nc.vector.tensor_copy(dview, ps[i][:])
                            else:
                                nc.scalar.copy(dview, ps[i][:])

            # V in natural [s, d] layout: stationary = vT seq-tile,
            # moving = W_v ht-block [128h, 512d] -> one bank per t-tile.
            for hf in range(2):
                psv = [psA.tile([128, DPC], F32, tag="psA",
                                name=f"psv{hf}_{i}") for i in range(8)]
                for ht in range(NHT):
                    vrow = xrow_pool.tile([128, SH], BF16, tag="xrow")
                    nc.sync.dma_start(
                        out=vrow[:],
                        in_=xT["v"][hf * HIDDEN + ht * 128:
                                    hf * HIDDEN + (ht + 1) * 128, :])
                    for sti in range(8):
                        nc.tensor.matmul(
                            psv[sti][:],
                            lhsT=vrow[:, sti * 128:(sti + 1) * 128],
                            rhs=w_sb["v"][:, ht * DPC:(ht + 1) * DPC],
                            start=(ht == 0), stop=(ht == NHT - 1))
                for sti in range(8):
                    st = hf * 8 + sti
                    dview = vh_sb[:, st * DPC:(st + 1) * DPC]
                    if sti % 2 == 0:
                        nc.vector.tensor_copy(dview, psv[sti][:])
                    else:
                        nc.scalar.copy(dview, psv[sti][:])

    coll_insts = []

    def phase_b():
        """Flat software pipeline over all (l, qb, g) groups: PE stream
        s(k), s(k+1), PV(k), s(k+2), PV(k+1), ... crosses iteration
        boundaries, so the scalar-engine EXP queue never drains and the
        PE never waits out a pipeline refill."""
        nonlocal cc_sb
        # cc reuses wq's SBUF space (same shape/dtype; wq is dead after
        # phase A - tile inserts the WAR dep on wq's readers)
        cc_sb = w_pool.tile([128, NHT * QBLK], BF16, tag="wq", name="cc")
        # wo prefetch: the sync queue is idle for all of phase B, so the
        # 8MB stream hides completely here without competing with rows.
        for ht in range(NHT):
            wo_chunk(ht)
        with tc.tile_pool(name="pssA", bufs=1, space="PSUM") as psA_pool, \
             tc.tile_pool(name="pssB", bufs=1, space="PSUM") as psB_pool, \
             tc.tile_pool(name="pspv", bufs=2, space="PSUM") as pv_pool:
            # l-outer: head l's AllToAll fires when its 4 q-blocks
            # finish - collectives 0..2 hide under phase B compute.
            groups = [(qb, l, g) for l in range(HPC) for qb in range(NQB)
                      for g in range(NG)]
            st = {}  # (qb, l) -> iteration state

            def state(qb, l):
                key = (qb, l)
                if key not in st:
                    st[key] = {
                        "pv": pv_pool.tile([128, QBLK], F32, tag="pv",
                                           name=f"pv{qb}_{l}"),
                        "ess": [None] * NG,
                        "acc": None,
                    }
                return st[key]

            def emit_scores(qb, l, g):
                it = state(qb, l)
                k0, n = GRP[g]
                pool = psA_pool if g % 2 == 0 else psB_pool
                w = pool.tile([128, 3 * QBLK], F32, tag="w",
                              name=f"wide{qb}_{l}_{g}")
                it[f"w{g}"] = w
                rhs_q = qh_sb[:, l * S + qb * QBLK: l * S + (qb + 1) * QBLK]
                for j in range(n):
                    kt = k0 + j
                    nc.tensor.matmul(
                        w[:, j * QBLK:(j + 1) * QBLK],
                        lhsT=kh_sb[:, l * S + kt * 128: l * S + (kt + 1) * 128],
                        rhs=rhs_q, start=True, stop=True)

            def emit_act(qb, l, g):
                it = state(qb, l)
                n = GRP[g][1]
                es = es_pool.tile([128, 3 * QBLK], BF16, tag="es",
                                  name=f"es{qb}_{l}_{g}")
                it["ess"][g] = es
                nc.scalar.activation(
                    es[:, :n * QBLK], it[f"w{g}"][:, :n * QBLK],
                    mybir.ActivationFunctionType.Exp, scale=scale)

            def emit_pv(qb, l, g):
                it = state(qb, l)
                k0, n = GRP[g]
                for j in range(n):
                    kt = k0 + j
                    nc.tensor.matmul(
                        it["pv"][:],
                        lhsT=vh_sb[:, kt * DPC + l * 128:
                                   kt * DPC + (l + 1) * 128],
                        rhs=it["ess"][g][:, j * QBLK:(j + 1) * QBLK],
                        start=(kt == 0), stop=(kt == NKT - 1))

            def emit_add(qb, l, g):
                it = state(qb, l)
                if g == 0 or g == 5:
                    return
                a = acc_pool.tile([128, 3 * QBLK], BF16, tag="acc",
                                  name=f"acc{qb}_{l}_{g}")
                if g == 1:
                    nc.vector.tensor_add(a[:], it["ess"][0][:],
                                         it["ess"][1][:])
                else:
                    nc.vector.tensor_add(a[:], it["acc"][:], it["ess"][g][:])
                it["acc"] = a
                # pre-fold the denominator off the critical path: after
                # g4's add the 1536-wide acc is final, so the 1536->512
                # folds run here and the post-act(g5) tail is ONE add.
                if g == 4:
                    f1 = fld_pool.tile([128, QBLK], BF16, tag="f1")
                    nc.vector.tensor_add(f1[:], a[:, 0:QBLK],
                                         a[:, QBLK:2 * QBLK])
                    f2 = fld_pool.tile([128, QBLK], BF16, tag="f2")
                    nc.vector.tensor_add(f2[:], f1[:], a[:, 2 * QBLK:3 * QBLK])
                    it["f2"] = f2

            def emit_tail(qb, l):
                # one fold add, PE ones-matmul partition reduce into the
                # freed psB wide slot, DVE reciprocal, normalize, scatter.
                it = state(qb, l)
                f3 = fld_pool.tile([128, QBLK], BF16, tag="f3")
                nc.vector.tensor_add(f3[:], it["f2"][:],
                                     it["ess"][5][:, 0:QBLK])
                rbp = psB_pool.tile([128, 3 * QBLK], F32, tag="w",
                                    name=f"rbp{qb}_{l}")
                nc.tensor.matmul(rbp[:, 0:QBLK], lhsT=ones_sb[:],
                                 rhs=f3[:], start=True, stop=True)
                rb = rb_pool.tile([128, QBLK], F32, tag="rb")
                nc.vector.reciprocal_approx_fast(rb[:], rbp[:, 0:QBLK])
                ao = ao_pool.tile([128, QBLK], BF16, tag="ao")
                nc.vector.tensor_mul(ao[:], it["pv"][:], rb[:])
                nc.gpsimd.dma_start(
                    out=a2a_in[l][:, qb * QBLK:(qb + 1) * QBLK], in_=ao[:])

            def emit_coll(l, tok_off):
                # end-of-head AllGather among this batch's 4 cores; the
                # trigger blocks the gpsimd queue until completion, which
                # only delays head l+1's (non-critical) fold work.
                coll = nc.gpsimd.collective_compute(
                    "AllGather", mybir.AluOpType.bypass,
                    replica_groups=RG,
                    ins=[a2a_in[l][:, :]], outs=[a2a_out[l][:, :]])
                coll_insts.append(coll)
                for gi in range(N_GRP):
                    dt = gi * HPC + l
                    src = AP(a2a_out[l].tensor,
                             tok_off + gi * 128 * S, [[S, 128], [1, QBLK]])
                    dma = nc.gpsimd.dma_start(
                        out=cc_sb[:, dt * QBLK:(dt + 1) * QBLK], in_=src)
                    tile.add_dep_helper(dma.ins, coll.ins,
                                        reason="a2a_out after collective")

            # my token-block column offset inside each AllGather block:
            # (pid % N_GRP) * QBLK, loaded once on the gpsimd engine
            tok_off = (nc.gpsimd.partition_id() % N_GRP) * QBLK

            emit_scores(*groups[0])
            for k, grp in enumerate(groups):
                if k + 1 < len(groups):
                    emit_scores(*groups[k + 1])
                emit_act(*grp)
                emit_pv(*grp)
                emit_add(*grp)
                if grp[2] == NG - 1:
                    emit_tail(grp[0], grp[1])
                    if grp[0] == NQB - 1:
                        emit_coll(grp[1], tok_off)

    # ================= Phase D: output projection ======================
    def phase_d():
        # contraction chunks from heads 0..2 first: collective #3 hides
        # under their 12 accumulation steps (psum accumulate commutes).
        DT_ORDER = [dt for l in range(HPC) for dt in (l, 4 + l, 8 + l, 12 + l)]
        DT_ORDER = ([dt for dt in DT_ORDER if dt % HPC != HPC - 1]
                    + [dt for dt in DT_ORDER if dt % HPC == HPC - 1])
        OBLK = 512
        NOB = HIDDEN // OBLK
        with tc.tile_pool(name="pso", bufs=8, space="PSUM") as pso_pool:
            for p in range(2):
                pso = [pso_pool.tile([128, OBLK], F32, tag="pso",
                                     name=f"pso{p}_{i}") for i in range(8)]
                for di, dt in enumerate(DT_ORDER):
                    for ti in range(2):
                        tt = p * 2 + ti
                        lhs = cc_sb[:, dt * QBLK + tt * 128:
                                    dt * QBLK + (tt + 1) * 128]
                        for ot in range(NOB):
                            nc.tensor.matmul(
                                pso[ti * NOB + ot][:], lhsT=lhs,
                                rhs=wo_sb[:, dt * HIDDEN + ot * OBLK:
                                          dt * HIDDEN + (ot + 1) * OBLK],
                                start=(di == 0), stop=(di == NHT - 1))
                for ti in range(2):
                    tt = p * 2 + ti
                    for ot in range(NOB):
                        osb = osb_pool.tile([128, OBLK], F32, tag="osb")
                        # alternate engines so the end-of-kernel drain
                        # runs two-wide.
                        if ot % 2 == 0:
                            nc.vector.tensor_copy(osb[:], pso[ti * NOB + ot][:])
                            eng = nc.scalar
                        else:
                            nc.scalar.copy(osb[:], pso[ti * NOB + ot][:])
                            eng = nc.sync
                        eng.dma_start(
                            out=out[tt * 128:(tt + 1) * 128,
                                    ot * OBLK:(ot + 1) * OBLK],
                            in_=osb[:])

    phase_a()
    phase_b()
    phase_d()


def build_nc(S: int):
    nc = bacc.Bacc("TRN2", target_bir_lowering=False, debug=False,
                   enable_asserts=False, num_devices=N_CORES)
    SH = S // 2
    QBLK = 512
    aps = {
        # half-major: [2 halves, HIDDEN, S/2] flattened - contiguous
        # 256KB half-row slabs for streaming reads
        "qT": nc.dram_tensor("qT", [2 * HIDDEN, SH], BF16,
                             kind="ExternalInput").ap(),
        "kT": nc.dram_tensor("kT", [2 * HIDDEN, SH], BF16,
                             kind="ExternalInput").ap(),
        "vT": nc.dram_tensor("vT", [2 * HIDDEN, SH], BF16,
                             kind="ExternalInput").ap(),
        "wqT": nc.dram_tensor("wqT", [128, NHT * DPC], BF16,
                              kind="ExternalInput").ap(),
        "wkT": nc.dram_tensor("wkT", [128, NHT * DPC], BF16,
                              kind="ExternalInput").ap(),
        "wvT": nc.dram_tensor("wvT", [128, NHT * DPC], BF16,
                              kind="ExternalInput").ap(),
        "woT": nc.dram_tensor("woT", [128, NHT * HIDDEN], BF16,
                              kind="ExternalInput").ap(),
        "out": nc.dram_tensor("out", [QBLK, HIDDEN], F32,
                              kind="ExternalOutput").ap(),
        "a2a_in": [nc.dram_tensor(f"a2a_in{l}", [128, S],
                                  BF16).ap() for l in range(HPC)],
        "a2a_out": [nc.dram_tensor(f"a2a_out{l}", [N_GRP * 128, S],
                                   BF16).ap() for l in range(HPC)],
    }
    with tile.TileContext(nc) as tc:
        with ExitStack() as ctx:
            _mha_kernel(ctx, tc, aps, S)
    nc.compile()
    return nc


_NC_CACHE: dict = {}


def _tile_weight(w_slice_T):
    """[H, D] -> [128, (H//128)*D] with 128-row tiles laid out consecutively."""
    H, D = w_slice_T.shape
    return np.ascontiguousarray(
        w_slice_T.reshape(H // 128, 128, D).transpose(1, 0, 2).reshape(
            128, (H // 128) * D))


def _half_major(x):
    """[S, H] -> [2*H, S/2]: transpose then split S into 2 contiguous halves."""
    S, H = x.shape
    return np.ascontiguousarray(
        x.T.reshape(H, 2, S // 2).transpose(1, 0, 2).reshape(
            2 * H, S // 2)).astype(NPBF16)


def make_in_maps(q, k, v, w_q, w_k, w_v, w_o):
    """Host-side shard/cast. Returns per-core input dicts."""
    qT = [_half_major(q[b]) for b in range(B)]
    kT = [_half_major(k[b]) for b in range(B)]
    vT = [_half_major(v[b]) for b in range(B)]
    woT = _tile_weight(np.ascontiguousarray(w_o.T).astype(NPBF16))
    wslices = []
    for g in range(N_GRP):
        d0 = g * DPC
        wslices.append({
            "wqT": _tile_weight(
                np.ascontiguousarray(w_q[d0:d0 + DPC, :].T).astype(NPBF16)),
            "wkT": _tile_weight(
                np.ascontiguousarray(w_k[d0:d0 + DPC, :].T).astype(NPBF16)),
            "wvT": _tile_weight(
                np.ascontiguousarray(w_v[d0:d0 + DPC, :].T).astype(NPBF16)),
        })
    in_maps = []
    for c in range(N_CORES):
        b, g = divmod(c, N_GRP)
        m = {"qT": qT[b], "kT": kT[b], "vT": vT[b], "woT": woT}
        m.update(wslices[g])
        in_maps.append(m)
    return in_maps


def kernel(q, k, v, mask, w_q, w_k, w_v, w_o, _trace=False):
    q = np.asarray(q, np.float32)
    k = np.asarray(k, np.float32)
    v = np.asarray(v, np.float32)
    mask = np.asarray(mask)
    w_q = np.asarray(w_q, np.float32)
    w_k = np.asarray(w_k, np.float32)
    w_v = np.asarray(w_v, np.float32)
    w_o = np.asarray(w_o, np.float32)
    S = q.shape[1]

    if not np.all(mask != 0):
        # General-mask fallback (never hit for the eval problem: mask is
        # all ones).  Computed on host for correctness.
        return _numpy_reference(q, k, v, mask, w_q, w_k, w_v, w_o)

    if S not in _NC_CACHE:
        _NC_CACHE[S] = build_nc(S)
    nc = _NC_CACHE[S]

    in_maps = make_in_maps(q, k, v, w_q, w_k, w_v, w_o)
    res = run_bass_kernel_spmd(nc, in_maps, core_ids=list(range(N_CORES)),
                               trace=_trace)

    QBLK = 512
    out = np.empty((B, S, HIDDEN), np.float32)
    for c in range(N_CORES):
        b, g = divmod(c, N_GRP)
        out[b, g * QBLK:(g + 1) * QBLK, :] = res.results[c]["out"]
    if _trace:
        return out, res
    return out


def _numpy_reference(q, k, v, mask, w_q, w_k, w_v, w_o):
    Bn, S, H = q.shape
    dk = H // HEADS

    def split_heads(x, w):
        y = x @ w.T
        return y.reshape(Bn, S, HEADS, dk).transpose(0, 2, 1, 3)

    qh = split_heads(q, w_q)
    kh = split_heads(k, w_k)
    vh = split_heads(v, w_v)
    s = np.einsum("bhqd,bhkd->bhqk", qh, kh) / np.sqrt(np.float32(dk))
    s = np.where(mask[:, None, :, :] == 0, np.float32(-1e9), s)
    s = s - s.max(-1, keepdims=True)
    e = np.exp(s)
    a = e / e.sum(-1, keepdims=True)
    o = np.einsum("bhqk,bhkd->bhqd", a, vh)
    o = o.transpose(0, 2, 1, 3).reshape(Bn, S, H)
    return (o @ w_o.T).astype(np.float32)
